# revision 23
# baseline (speedup 1.0000x reference)
"""Bass/Trainium2 kernel for attention-LSTM decoder (nn_Attention_49289044688898).

Data-parallel over batch: 512 rows -> 8 NeuronCores x 64 rows. Weights replicated.
Per core, per decode step s (26 steps):
  q   = h @ Wh                                  (PE, bh folded into Hproj)
  arg = HprojT + qT (broadcast over t)          (DVE, bf16 2x)
  th  = tanh(arg)                               (ACT)
  e   = sum_h Ws[h]*th[h, t, b]                 (PE, Ws stationary M=1)
  alpha = softmax_t(e)                          (DVE/ACT small)
  ctx = sum_t alpha[b,t]*batch_H[b,t,c]         (PE, block-diag alpha stationary)
  z   = ctx @ Kc + h @ R + onehot @ Ko'         (PE; Ko' has lstm_bias folded)
  gates/c/h                                     (ACT/DVE)
  probs[:, s, :] = h @ Wgen + bgen              (PE)
Layouts:
  attention world: [128 part = h_lo, 4 h_hi, 64 t, 64 b]   (h = h_hi*128 + h_lo)
  context world:   [128 part = (b%2)*64 + t, 32 kt=b//2, 512 c]
  LSTM world:      [64 part = b, free]
Transposes between worlds via DMA xbar (bf16).
"""

import os
import numpy as np
import ml_dtypes
from contextlib import ExitStack

B, T, C, H, NCC, S = 512, 64, 512, 512, 96, 26
NCORES = 8
BS = B // NCORES          # 64 batch rows per core
BF = ml_dtypes.bfloat16

_CACHE = {}


def build_bass():
    import concourse.bass as bass
    import concourse.bacc as bacc
    import concourse.tile as tile
    import concourse.mybir as mybir

    f32 = mybir.dt.float32
    bf16 = mybir.dt.bfloat16
    AF = mybir.ActivationFunctionType
    AX = mybir.AxisListType

    nc = bacc.Bacc("TRN2", target_bir_lowering=False)

    # ---- DRAM I/O ----
    bHT_d = nc.dram_tensor("bHT", [C, T, BS], bf16, kind="ExternalInput")      # [c, t, b]
    bHc_d = nc.dram_tensor("bHc", [BS // 2, 128, C], bf16, kind="ExternalInput")  # [kt, (b2 t), c]
    wi_d = nc.dram_tensor("wi", [C, H], bf16, kind="ExternalInput")
    wh_d = nc.dram_tensor("wh", [H, H], bf16, kind="ExternalInput")
    bh_d = nc.dram_tensor("bh", [128, 4], f32, kind="ExternalInput")           # bh chunks [h_lo, h_hi]
    ws_d = nc.dram_tensor("ws", [128, 4, 32], bf16, kind="ExternalInput")      # Ws chunks x32 cols
    kc_d = nc.dram_tensor("kc", [C, 4 * H], bf16, kind="ExternalInput")
    rr_d = nc.dram_tensor("rr", [H, 4 * H], bf16, kind="ExternalInput")
    ko_d = nc.dram_tensor("ko", [NCC, 4 * H], bf16, kind="ExternalInput")      # + lstm_bias folded
    oh_d = nc.dram_tensor("oh", [NCC, S, BS], bf16, kind="ExternalInput")      # one-hot transposed
    wg_d = nc.dram_tensor("wg", [H, NCC], bf16, kind="ExternalInput")
    bg_d = nc.dram_tensor("bg", [BS, NCC], f32, kind="ExternalInput")          # bgen replicated
    out_d = nc.dram_tensor("out", [BS, S, NCC], f32, kind="ExternalOutput")
    escr_d = nc.dram_tensor("escr", [S, T * BS], f32)  # internal scratch for e scatter

    with tile.TileContext(nc) as tc, ExitStack() as ctx:
        # ---- pools ----
        big = ctx.enter_context(tc.tile_pool(name="big", bufs=1))       # 32KB/part tensors
        wpool = ctx.enter_context(tc.tile_pool(name="wpool", bufs=1))   # weights
        small = ctx.enter_context(tc.tile_pool(name="small", bufs=2))   # per-step small tiles
        tiny = ctx.enter_context(tc.tile_pool(name="tiny", bufs=4))     # [64,1]-ish
        gates = ctx.enter_context(tc.tile_pool(name="gates", bufs=4))   # gate/tmp tiles
        state = ctx.enter_context(tc.tile_pool(name="state", bufs=2))   # h, c, hT
        pz = ctx.enter_context(tc.tile_pool(name="pz", bufs=3, space="PSUM"))
        pe_pool = ctx.enter_context(tc.tile_pool(name="pe", bufs=2, space="PSUM"))
        ppr = ctx.enter_context(tc.tile_pool(name="ppr", bufs=2, space="PSUM"))

        dma = nc.sync

        # ---- load weights/SBUF-resident tensors ----
        bHc = big.tile([128, BS // 2, C], bf16, tag="bHc")
        dma.dma_start(out=bHc, in_=bHc_d[:].rearrange("k p c -> p k c"))

        # batch_H^T for Hproj matmul (freed after prolog; shares slot with tanh buffer)
        bHT = big.tile([128, 4, T * BS], bf16, tag="thbuf")
        dma.dma_start(out=bHT, in_=bHT_d[:].rearrange("(ch cl) t b -> cl ch (t b)", cl=128))

        wi = wpool.tile([128, 4, H], bf16, tag="wi")
        dma.dma_start(out=wi, in_=wi_d[:].rearrange("(ch cl) h -> cl ch h", cl=128))
        wh = wpool.tile([128, 4, H], bf16, tag="wh")
        dma.dma_start(out=wh, in_=wh_d[:].rearrange("(hh hl) h -> hl hh h", hl=128))
        bh = wpool.tile([128, 4], f32, tag="bh")
        dma.dma_start(out=bh, in_=bh_d[:])
        ws = wpool.tile([128, 4, 32], bf16, tag="ws")
        dma.dma_start(out=ws, in_=ws_d[:])
        kc = wpool.tile([128, 4, 4 * H], bf16, tag="kc")
        dma.dma_start(out=kc, in_=kc_d[:].rearrange("(kh kl) n -> kl kh n", kl=128))
        rr = wpool.tile([128, 4, 4 * H], bf16, tag="rr")
        dma.dma_start(out=rr, in_=rr_d[:].rearrange("(kh kl) n -> kl kh n", kl=128))
        ko = wpool.tile([NCC, 4 * H], bf16, tag="ko")
        dma.dma_start(out=ko, in_=ko_d[:])
        oh = wpool.tile([NCC, S, BS], bf16, tag="oh")
        dma.dma_start(out=oh, in_=oh_d[:])
        wg = wpool.tile([128, 4, NCC], bf16, tag="wg")
        dma.dma_start(out=wg, in_=wg_d[:].rearrange("(hh hl) n -> hl hh n", hl=128))
        bg = wpool.tile([BS, NCC], f32, tag="bg")
        dma.dma_start(out=bg, in_=bg_d[:])

        # block-diag alpha holder: [part=(b2,t), kt, col=b] ; zero once, nonzero slots rewritten per step
        ablk = wpool.tile([128, BS // 2, BS], bf16, tag="ablk")
        nc.vector.memset(ablk, 0.0)

        # initial state (zeros) + padding init
        hT = state.tile([128, 4, BS], bf16, tag="hT")
        nc.vector.memset(hT, 0.0)
        c_st = state.tile([BS, H], f32, tag="c")
        nc.vector.memset(c_st, 0.0)
        alpha_pad = small.tile([BS, 128], bf16, tag="alphapad")
        nc.vector.memset(alpha_pad, 0.0)



        # ---- prolog: HprojT = (batch_H @ Wi)^T + bh, layout [h_lo, h_hi, (t b)] ----
        hprojT = big.tile([128, 4, T * BS], bf16, tag="hprojT")
        NCHUNK = T * BS // 512  # 8
        for m in range(4):
            for n in range(NCHUNK):
                ps = pz.tile([128, 512], f32, tag="pz")
                for k in range(4):
                    nc.tensor.matmul(
                        ps,
                        wi[:, k, m * 128:(m + 1) * 128],
                        bHT[:, k, n * 512:(n + 1) * 512],
                        start=(k == 0), stop=(k == 3),
                    )
                nc.scalar.activation(
                    out=hprojT[:, m, n * 512:(n + 1) * 512], in_=ps,
                    func=AF.Identity, bias=bh[:, m:m + 1], scale=1.0,
                )

        import concourse.bass as _b

        def bcast_t(ap3):
            # [128, 64(b)] -> [128, 64(t,step0), 64(b)]
            return _b.AP(tensor=ap3.tensor, offset=ap3.offset,
                         ap=[ap3.ap[0], [0, T], ap3.ap[1]])

        # ---- decode steps ----
        for s in range(S):
            # q = h @ Wh  -> psum [64, 512] f32
            pq = pz.tile([BS, H], f32, tag="pz")
            for k in range(4):
                nc.tensor.matmul(pq, hT[:, k, :], wh[:, k, :],
                                 start=(k == 0), stop=(k == 3))
            q_sb = small.tile([BS, H], bf16, tag="q_sb")
            nc.scalar.copy(q_sb, pq)
            qT = small.tile([128, 4, BS], bf16, tag="qT")
            for k in range(4):
                dma.dma_start(out=qT[:, k, :], in_=q_sb[:, k * 128:(k + 1) * 128],
                              transpose=True)

            # arg = HprojT + qT  (broadcast over t), tanh in-place
            th = big.tile([128, 4, T * BS], bf16, tag="thbuf")
            for k in range(4):
                nc.vector.tensor_add(th[:, k, :].rearrange("p (t b) -> p t b", t=T),
                                     hprojT[:, k, :].rearrange("p (t b) -> p t b", t=T),
                                     bcast_t(qT[:, k, :]))
                nc.scalar.activation(out=th[:, k, :], in_=th[:, k, :], func=AF.Tanh)

            # e[t,b] = sum_h ws*th via PE (M=1). 8 chunks land on partitions
            # {0,32,64,96} of 2 psum tiles via col-group tile_position, then
            # 2 strided DVE copies + 2 DMAs to DRAM scratch + 1 gather DMA.
            e_sb = small.tile([BS, T], f32, tag="e_sb")
            for half in range(2):
                pe = pe_pool.tile([128, 512], f32, tag="pe")
                for j in range(4):
                    n = half * 4 + j
                    bp = 32 * j
                    for k in range(4):
                        nc.tensor.matmul(pe[bp:bp + 32, :], ws[:, k, :],
                                         th[:, k, n * 512:(n + 1) * 512],
                                         start=(k == 0), stop=(k == 3),
                                         tile_position=(0, bp))
                est = small.tile([128, 512], f32, tag="e_stage")
                nc.vector.tensor_copy(est, pe)
                est_ap = est[:]
                src = _b.AP(tensor=est_ap.tensor, offset=est_ap.offset,
                            ap=[[est_ap.ap[0][0] * 32, 4], est_ap.ap[1]])
                dma.dma_start(out=escr_d[s, half * 2048:(half + 1) * 2048],
                              in_=src)
            esl = escr_d[s, :]
            src = _b.AP(tensor=esl.tensor, offset=esl.offset,
                        ap=[[1, BS], [BS, T]])
            dma.dma_start(out=e_sb, in_=src)

            # softmax over t
            mx = tiny.tile([BS, 1], f32, tag="mx")
            nc.vector.reduce_max(mx, e_sb, axis=AX.X)
            nmx = tiny.tile([BS, 1], f32, tag="nmx")
            nc.vector.tensor_scalar_mul(nmx, mx, -1.0)
            ex = small.tile([BS, T], f32, tag="ex")
            nc.scalar.activation(out=ex, in_=e_sb, func=AF.Exp, bias=nmx, scale=1.0)
            sm = tiny.tile([BS, 1], f32, tag="sm")
            nc.vector.reduce_sum(sm, ex, axis=AX.X)
            rcp = tiny.tile([BS, 1], f32, tag="rcp")
            nc.vector.reciprocal(rcp, sm)
            nc.vector.tensor_scalar_mul(alpha_pad[:, 0:T], ex, rcp)
            alphaT = small.tile([128, BS], bf16, tag="alphaT")
            dma.dma_start(out=alphaT, in_=alpha_pad, transpose=True)

            # scatter alphaT[t, b] into ablk[(b2 t), kt, col=b]: 2 DMAs (even/odd b)
            aT = alphaT[:]
            for par in (0, 1):
                src = _b.AP(tensor=aT.tensor, offset=aT.offset + par * aT.ap[1][0],
                            ap=[[aT.ap[0][0], T], [2 * aT.ap[1][0], BS // 2]])
                ab = ablk[:]
                dst = _b.AP(tensor=ab.tensor,
                            offset=ab.offset + par * (64 * ab.ap[0][0] + ab.ap[2][0]),
                            ap=[[ab.ap[0][0], T], [ab.ap[1][0] + 2 * ab.ap[2][0], BS // 2]])
                dma.dma_start(out=dst, in_=src)

            # ctx[b, c] = sum over (b2,t) blocks
            pctx = pz.tile([BS, C], f32, tag="pz")
            for kt in range(BS // 2):
                nc.tensor.matmul(pctx, ablk[:, kt, :], bHc[:, kt, :],
                                 start=(kt == 0), stop=(kt == BS // 2 - 1))
            ctx_sb = small.tile([BS, C], bf16, tag="ctx_sb")
            nc.scalar.copy(ctx_sb, pctx)
            xTc = small.tile([128, 4, BS], bf16, tag="xTc")
            for k in range(4):
                dma.dma_start(out=xTc[:, k, :], in_=ctx_sb[:, k * 128:(k + 1) * 128],
                              transpose=True)

            # z chunks in gate order f, i, g, o ; z = x@Kc + h@R + onehot@Ko'
            gate_sl = {"i": 0, "f": 1, "g": 2, "o": 3}
            sig = {}
            for gname in ("f", "i", "g", "o"):
                gsl = slice(gate_sl[gname] * 512, (gate_sl[gname] + 1) * 512)
                pzt = pz.tile([BS, 512], f32, tag="pz")
                for k in range(4):
                    nc.tensor.matmul(pzt, xTc[:, k, :], kc[:, k, gsl],
                                     start=(k == 0), stop=False)
                for k in range(4):
                    nc.tensor.matmul(pzt, hT[:, k, :], rr[:, k, gsl],
                                     start=False, stop=False)
                nc.tensor.matmul(pzt, oh[:, s, :], ko[:, gsl],
                                 start=False, stop=True)
                g_sb = gates.tile([BS, 512], f32, tag="gate")
                if gname == "g":
                    nc.scalar.activation(out=g_sb, in_=pzt, func=AF.Tanh)
                else:
                    # sigmoid(x) = 0.5*tanh(0.5x) + 0.5 (keeps one ACT table set)
                    nc.scalar.activation(out=g_sb, in_=pzt, func=AF.Tanh, scale=0.5)
                    nc.vector.tensor_scalar(out=g_sb, in0=g_sb,
                                            scalar1=0.5, scalar2=0.5,
                                            op0=mybir.AluOpType.mult,
                                            op1=mybir.AluOpType.add)
                sig[gname] = g_sb

            # c = f*c + i*tanh(g); h = o*tanh(c)
            t1 = gates.tile([BS, H], f32, tag="tmp")
            nc.vector.tensor_mul(t1, sig["f"], c_st)
            t2 = gates.tile([BS, H], f32, tag="tmp")
            nc.vector.tensor_mul(t2, sig["i"], sig["g"])
            c_st = state.tile([BS, H], f32, tag="c")
            nc.vector.tensor_add(c_st, t1, t2)
            tc_sb = gates.tile([BS, H], f32, tag="tmp")
            nc.scalar.activation(out=tc_sb, in_=c_st, func=AF.Tanh)
            h_bf = small.tile([BS, H], bf16, tag="h_bf")
            nc.vector.tensor_mul(h_bf, sig["o"], tc_sb)
            hT = state.tile([128, 4, BS], bf16, tag="hT")
            for k in range(4):
                dma.dma_start(out=hT[:, k, :], in_=h_bf[:, k * 128:(k + 1) * 128],
                              transpose=True)

            # probs[:, s, :] = h @ Wgen + bgen
            pp = ppr.tile([BS, NCC], f32, tag="pp")
            for k in range(4):
                nc.tensor.matmul(pp, hT[:, k, :], wg[:, k, :],
                                 start=(k == 0), stop=(k == 3))
            pr_sb = small.tile([BS, NCC], f32, tag="pr_sb")
            nc.vector.tensor_add(pr_sb, pp, bg)
            dma.dma_start(out=out_d[:, s, :], in_=pr_sb)

    nc.finalize()
    return nc


def _prep_core(inputs, i):
    bsl = slice(i * BS, (i + 1) * BS)
    bh_i = np.asarray(inputs["batch_H"][bsl], np.float32)          # [64, 64, 512]
    text_i = np.asarray(inputs["text"][bsl])                       # [64, 26]
    m = {}
    m["bHT"] = np.ascontiguousarray(bh_i.transpose(2, 1, 0)).astype(BF)
    m["bHc"] = np.ascontiguousarray(bh_i.reshape(BS // 2, 128, C)).astype(BF)
    m["wi"] = np.asarray(inputs["Wi"], np.float32).astype(BF)
    m["wh"] = np.asarray(inputs["Wh"], np.float32).astype(BF)
    m["bh"] = np.ascontiguousarray(
        np.asarray(inputs["bh"], np.float32).reshape(4, 128).T)
    wsr = np.ascontiguousarray(
        np.asarray(inputs["Ws"], np.float32)[:, 0].reshape(4, 128).T).astype(BF)
    m["ws"] = np.repeat(wsr[:, :, None], 32, axis=2)
    lk = np.asarray(inputs["lstm_kernel"], np.float32)
    lb = np.asarray(inputs["lstm_bias"], np.float32)
    m["kc"] = lk[:C].astype(BF)
    m["ko"] = (lk[C:] + lb[None, :]).astype(BF)
    m["rr"] = np.asarray(inputs["lstm_rec"], np.float32).astype(BF)
    m["oh"] = (np.arange(NCC)[:, None, None] == text_i.T[None, :, :]).astype(BF)
    m["wg"] = np.asarray(inputs["Wgen"], np.float32).astype(BF)
    m["bg"] = np.tile(np.asarray(inputs["bgen"], np.float32)[None, :], (BS, 1))
    return m


def kernel(_trace=False, **inputs):
    from concourse import bass_utils
    if "nc" not in _CACHE:
        _CACHE["nc"] = build_bass()
    nc = _CACHE["nc"]
    in_maps = [_prep_core(inputs, i) for i in range(NCORES)]
    res = bass_utils.run_bass_kernel_spmd(nc, in_maps, list(range(NCORES)),
                                          trace=_trace)
    _CACHE["last_result"] = res
    out = np.concatenate([r["out"] for r in res.results], axis=0)
    return out.astype(np.float32)
